# revision 34
# baseline (speedup 1.0000x reference)
"""Trainium2 Bass kernel for a custom attention block (qkv-proj + LN(q,k) +
RoPE + causal attention + out-proj), distributed over 8 NeuronCores.

Sharding: 2 cores per batch (B=4). Core role r=c%2 takes q-token blocks
{0,3} (r=0) or {1,2} (r=1) of 512 tokens; every core computes K/V for the
full 2048-token sequence of its batch (no collectives). The compiled
program is identical on all cores; per-core differences are input data
only. To keep the q-slab offsets compile-time-constant, each core sees
the sequence in a per-role BLOCK PERMUTATION (r=0: 0,1,2,3; r=1:
1,0,3,2), so its q blocks always sit at permuted positions {0,3} and the
causal diagonal lands on the same slot indices for both roles. The
cos/sin tables, causal masks, exp row-biases and output assembly are
permutation-aware host data.

All matmuls run in bf16 (same PE rate as fp32r but faster weight loads,
half the DMA/SBUF), with fp32 PSUM accumulation. x is SBUF-resident; q
stays SBUF-resident post-rope; k round-trips through DRAM feature-major;
v is computed feature-major (so the PE reuses each stationary weight
tile across 4 moving tiles) and transposed to token-major on the fly by
DMA-transpose loads during attention.

Engine split: PE does projections/scores/PV; Scalar does exp, squares
and PSUM->SBUF copies; DVE does LN/rope muls, masks and half the
softmax-denominator accumulation; GpSimd does the other half plus the
rope add and LN sumsq chains. Softmax denominator = chained elementwise
adds of the exp tiles + one ones-matmul partition reduction per head.
"""

import math

import numpy as np

import concourse.bass as bass
import concourse.mybir as mybir
import concourse.tile as tile
from concourse import bacc
from concourse.bass import ds

F32 = mybir.dt.float32
F32R = mybir.dt.float32r
BF16 = mybir.dt.bfloat16
AF = mybir.ActivationFunctionType
OP = mybir.AluOpType

P = 128
HD = 128
D = 2048
S = 2048
NH = D // HD          # 16 heads = feature chunks
DC = D // P           # 16 contraction chunks
NQTOK = 1024          # q tokens per core
QT = 512              # q/attention tile width (moving dim)
NQ = NQTOK // QT      # 2 q tiles per core
EXP_BIAS = 8.0
EPS = 1e-5
SLOTS = (8, 16)       # kv 128-chunks per q tile (max over the two roles)
# elementwise masks only on the diagonal slots (identical for both roles
# thanks to the block permutation); everything else is handled by the
# per-row exp bias (-EXP_BIAS valid / BIAS_INVALID invalid).
MASKED = ((0, 1, 2, 3), (12, 13, 14, 15))
MAXM = 4
BIAS_INVALID = -30.0
Q_POS = (0, 3)        # structural (permuted) block positions of q slabs
KC = S // P           # 16 kv chunks
LOOKAHEAD = 2         # attention score-slot software pipeline depth


def _r(ap):
    """fp32 -> fp32r view for matmul operands."""
    return ap.bitcast(F32R)


def _v3(ap):
    """[P, n*128] AP -> [P, n, 128] view (avoids 1-free-dim DMA splits)."""
    return ap.rearrange("p (a x) -> p a x", x=P)


def build_program():
    nc = bacc.Bacc("TRN2", target_bir_lowering=False, debug=False)

    # ---- I/O ----
    xT_i = nc.dram_tensor("xT", [D, S], BF16, kind="ExternalInput").ap()
    wqk_i = nc.dram_tensor("wqk", [2 * NH, P, DC, P], BF16,
                           kind="ExternalInput").ap()
    wv_i = nc.dram_tensor("wv", [DC, P, NH, P], BF16,
                          kind="ExternalInput").ap()
    wo_i = nc.dram_tensor("wo", [NH, P, NH, P], BF16,
                          kind="ExternalInput").ap()
    cos_i = nc.dram_tensor("cos", [HD, S], BF16, kind="ExternalInput").ap()
    sin_i = nc.dram_tensor("sin", [HD, S], BF16, kind="ExternalInput").ap()
    gq_i = nc.dram_tensor("gq", [P, NH], F32, kind="ExternalInput").ap()
    bq_i = nc.dram_tensor("bq", [P, NH], F32, kind="ExternalInput").ap()
    gk_i = nc.dram_tensor("gk", [P, NH], F32, kind="ExternalInput").ap()
    bk_i = nc.dram_tensor("bk", [P, NH], F32, kind="ExternalInput").ap()
    masks_i = nc.dram_tensor("masks", [NQ, P, MAXM, QT], BF16,
                             kind="ExternalInput").ap()
    biast_i = nc.dram_tensor("biast", [P, NQ, 16], F32,
                             kind="ExternalInput").ap()
    onesc_i = nc.dram_tensor("onesc", [P, 1], F32, kind="ExternalInput").ap()
    onesr_i = nc.dram_tensor("onesr", [1, P], F32, kind="ExternalInput").ap()
    rotm_i = nc.dram_tensor("rotm", [P, P], BF16, kind="ExternalInput").ap()
    out_t = nc.dram_tensor("out", [D, NQTOK], F32, kind="ExternalOutput").ap()

    with tile.TileContext(nc) as tc:
        import contextlib

        ctx = contextlib.ExitStack()
        with ctx:
            sb = ctx.enter_context(tc.tile_pool(name="sb", bufs=1))
            psum = ctx.enter_context(tc.tile_pool(name="ps", bufs=1, space="PSUM"))
            dram = ctx.enter_context(tc.tile_pool(name="dram", bufs=1, space="DRAM"))

            # ---- DRAM scratch ----
            kts = dram.tile([P, NH, S], BF16, tag="kts", name="kts")
            vT = dram.tile([D, S], BF16, tag="vT", name="vT")

            # ---- constants / small inputs ----
            ones_col = sb.tile([P, 1], F32, tag="ones_col", name="ones_col")
            nc.sync.dma_start(_r(ones_col), _r(onesc_i))
            ones_row = sb.tile([1, P], F32, tag="ones_row", name="ones_row")
            nc.sync.dma_start(_r(ones_row), _r(onesr_i))
            eps1 = sb.tile([1, 1], F32, tag="eps1", name="eps1")
            nc.vector.memset(eps1, EPS)
            zero1 = sb.tile([1, 1], F32, tag="zero1", name="zero1")
            nc.vector.memset(zero1, 0.0)
            biast = sb.tile([P, NQ, 16], F32, tag="biast", name="biast")
            nc.sync.dma_start(biast, biast_i)
            rotm = sb.tile([P, P], BF16, tag="rotm", name="rotm")
            nc.sync.dma_start(rotm, rotm_i)
            gq = sb.tile([P, NH], F32, tag="gq", name="gq")
            nc.sync.dma_start(gq, gq_i)
            bq = sb.tile([P, NH], F32, tag="bq", name="bq")
            nc.sync.dma_start(bq, bq_i)
            gk = sb.tile([P, NH], F32, tag="gk", name="gk")
            nc.sync.dma_start(gk, gk_i)
            bk = sb.tile([P, NH], F32, tag="bk", name="bk")
            nc.sync.dma_start(bk, bk_i)
            cos_t = sb.tile([HD, S], BF16, tag="cos_t", name="cos_t")
            nc.sync.dma_start(_v3(cos_t), _v3(cos_i))
            sin_t = sb.tile([HD, S], BF16, tag="sin_t", name="sin_t")
            nc.sync.dma_start(_v3(sin_t), _v3(sin_i))

            # ---- resident x: [128, DC, S] bf16 (64KB/partition) ----
            xsb = sb.tile([P, DC, S], BF16, tag="xsb", bufs=1, name="xsb")
            for d in range(DC):
                nc.sync.dma_start(xsb[:, d], xT_i[ds(d * P, P), :])

            # ---- resident q (post-LN+rope): [128, NH, NQTOK] bf16 ----
            q_res = sb.tile([P, NH, NQTOK], BF16, tag="q_res", bufs=1,
                            name="q_res")

            def proj_group(ec_base, slab_offs, q_dst_offs, g_sb, b_sb):
                """Project x -> feature-partition [128, QT] tiles for each
                token slab in this group. Emits the matmul phase and
                returns a closure that emits the LN + rope tail (so the
                caller can interleave it into the next group's matmul
                stream and keep the PE dense).

                slab_offs: compile-time token offsets into the permuted
                sequence (index into x and cos/sin). If q_dst_offs is not
                None the result lands at q_res[:, :, q_dst_off]; else it
                DMAs to kts[:, :, slab_off].
                """
                n_s = len(slab_offs)
                holds = []
                for i in range(n_s):
                    if q_dst_offs is not None:
                        holds.append(q_res[:, :, ds(q_dst_offs[i], QT)])
                    else:
                        h = sb.tile([P, NH, QT], BF16, tag="khold", bufs=2,
                                    name="khold")
                        holds.append(h)
                sqsums = []
                for i in range(n_s):
                    sqsums.append(sb.tile([P, QT], F32, tag="acc", bufs=3,
                                          name="sqsum"))
                for ec in range(NH):
                    w = sb.tile([P, DC, P], BF16, tag="w", bufs=2, name="w")
                    nc.sync.dma_start(w, wqk_i[ec_base + ec])
                    pss = [psum.tile([P, QT], F32, tag="mm", bufs=5, name="ps")
                           for _ in range(n_s)]
                    for d in range(DC):
                        for i in range(n_s):
                            nc.tensor.matmul(
                                pss[i],
                                lhsT=w[:, d],
                                rhs=xsb[:, d, ds(slab_offs[i], QT)],
                                start=(d == 0),
                                stop=(d == DC - 1),
                            )
                    for i in range(n_s):
                        nc.scalar.copy(holds[i][:, ec], pss[i])
                        sq = sb.tile([P, QT], BF16, tag="sq", bufs=2,
                                     name="sq")
                        nc.scalar.square(sq, pss[i])
                        if ec == 0:
                            nc.vector.tensor_copy(_r(sqsums[i]), sq)
                        else:
                            nc.gpsimd.tensor_tensor(_r(sqsums[i]), sqsums[i],
                                                    sq, op=OP.add)

                def tail():
                    for i in range(n_s):
                        hold = holds[i]
                        csl = ds(slab_offs[i], QT)
                        # per-token sumsq: partition-sum via ones-matmul
                        pstat = psum.tile([1, QT], F32, tag="stat", bufs=3,
                                          name="pstat")
                        nc.tensor.matmul(pstat, lhsT=_r(ones_col),
                                         rhs=_r(sqsums[i]))
                        # rsig = exp(-0.5 * ln(sumsq/D + eps))
                        lnv = sb.tile([1, QT], F32, tag="stats_sb", bufs=4,
                                      name="lnv")
                        nc.scalar.activation(lnv, pstat, AF.Ln,
                                             scale=1.0 / D, bias=eps1)
                        rsig = sb.tile([1, QT], F32, tag="stats_sb", bufs=4,
                                       name="rsig")
                        nc.scalar.activation(_r(rsig), lnv, AF.Exp,
                                             bias=zero1, scale=-0.5)
                        ps_rep = psum.tile([P, QT], F32, tag="mm", bufs=5,
                                           name="ps_rep")
                        nc.tensor.matmul(ps_rep, lhsT=_r(ones_row),
                                         rhs=_r(rsig))
                        # pass 1: LN apply on all chunks (DVE)
                        for ec in range(NH):
                            ch = hold[:, ec]
                            nc.vector.tensor_tensor(ch, ch, ps_rep,
                                                    op=OP.mult)
                            nc.vector.tensor_scalar(
                                ch, ch,
                                scalar1=g_sb[:, ds(ec, 1)],
                                scalar2=b_sb[:, ds(ec, 1)],
                                op0=OP.mult, op1=OP.add,
                            )
                        # pass 2: rope; rotation matmuls stream back-to-back
                        for ec in range(NH):
                            ch = hold[:, ec]
                            ps_rot = psum.tile([P, QT], F32, tag="mm",
                                               bufs=5, name="ps_rot")
                            nc.tensor.matmul(ps_rot, lhsT=rotm, rhs=ch)
                            tmp = sb.tile([P, QT], BF16, tag="rtmp", bufs=2,
                                          name="rtmp")
                            nc.vector.tensor_tensor(tmp, ps_rot,
                                                    sin_t[:, csl], op=OP.mult)
                            nc.vector.tensor_tensor(ch, ch, cos_t[:, csl],
                                                    op=OP.mult)
                            nc.gpsimd.tensor_tensor(ch, ch, tmp, op=OP.add)
                        if q_dst_offs is None:
                            nc.sync.dma_start(
                                kts[:, :, ds(slab_offs[i], QT)], hold
                            )

                return tail

            def v_chunk(f):
                """Phase V chunk: w-stationary (reused across 4 moving
                tiles), writes v^T feature-major to DRAM."""
                wvf = sb.tile([P, DC, P], BF16, tag="w", bufs=2, name="wvf")
                nc.sync.dma_start(wvf, wv_i[:, :, f, :].rearrange(
                    "d p j -> p d j"))
                psv = [psum.tile([P, QT], F32, tag="mm", bufs=5, name="psv")
                       for _ in range(4)]
                for d in range(DC):
                    for ts in range(4):
                        nc.tensor.matmul(
                            psv[ts],
                            lhsT=wvf[:, d],
                            rhs=xsb[:, d, ds(ts * QT, QT)],
                            start=(d == 0),
                            stop=(d == DC - 1),
                        )
                for ts in range(4):
                    vtsb = sb.tile([P, QT], BF16, tag="vsb", bufs=3,
                                   name="vtsb")
                    nc.scalar.copy(vtsb, psv[ts])
                    nc.gpsimd.dma_start(
                        _v3(vT[ds(f * P, P), ds(ts * QT, QT)]), _v3(vtsb)
                    )

            # ---- Projections: each group's LN/rope tail is emitted
            # inside the NEXT group's matmul stream so the PE stays dense.
            tail_q = proj_group(0, [Q_POS[0] * QT, Q_POS[1] * QT], [0, QT],
                                gq, bq)
            pending_tail = tail_q
            for g in range(4):
                t_k = proj_group(NH, [g * QT], None, gk, bk)
                pending_tail()
                pending_tail = t_k
            v_chunk(0)
            pending_tail()
            for f in range(1, NH):
                v_chunk(f)

            # ---- Attention + out-projection per q tile ----
            def outproj_chunk(ot_src, qoff, e):
                """One out-projection feature chunk; pure PE work used to
                fill exp-latency gaps in the attention stream."""
                wot = sb.tile([P, NH, P], BF16, tag="wot", bufs=2,
                              name="wot")
                nc.sync.dma_start(wot, wo_i[e])
                psf = psum.tile([P, QT], F32, tag="mm", bufs=5,
                                name="psf")
                for h in range(NH):
                    nc.tensor.matmul(
                        psf,
                        lhsT=wot[:, h],
                        rhs=ot_src[:, h],
                        start=(h == 0),
                        stop=(h == NH - 1),
                    )
                fsb = sb.tile([P, QT], F32, tag="fsb", bufs=2,
                              name="fsb")
                nc.scalar.copy(fsb, psf)
                nc.sync.dma_start(
                    _v3(out_t[ds(e * P, P), ds(qoff, QT)]), _v3(fsb)
                )

            prev_ot = None
            for t in range(NQ):
                qsl_off = t * QT
                n_slots = SLOTS[t]
                # masks overlay the (now dead) cos buffer
                mt = sb.tile([P, MAXM, QT], BF16, tag="cos_t", bufs=1,
                             name="mt")
                nc.sync.dma_start(mt, masks_i[t])
                mpos = {kc: i for i, kc in enumerate(MASKED[t])}
                ot_res = sb.tile([P, NH, QT], BF16, tag="khold", bufs=2,
                                 name="ot_res")
                pending = None

                def finish_norm(pending):
                    psout_p, esum_p, h_p = pending
                    psden = psum.tile([1, QT], F32, tag="stat", bufs=3,
                                      name="psden")
                    nc.tensor.matmul(psden, lhsT=_r(ones_col), rhs=_r(esum_p))
                    rec0 = sb.tile([1, QT], F32, tag="stats_sb", bufs=4,
                                   name="rec0")
                    with nc.allow_low_precision(
                        reason="denominator reciprocal, 18 bits is plenty"
                    ):
                        nc.vector.reciprocal_approx_fast(rec0, psden)
                    rec = sb.tile([1, QT], F32, tag="stats_sb", bufs=4,
                                  name="rec")
                    nc.vector.tensor_copy(_r(rec), rec0)
                    psr = psum.tile([P, QT], F32, tag="mm", bufs=5,
                                    name="psr")
                    nc.tensor.matmul(psr, lhsT=_r(ones_row), rhs=_r(rec))
                    nc.vector.tensor_copy(ot_res[:, h_p], psout_p)
                    nc.vector.tensor_tensor(ot_res[:, h_p], ot_res[:, h_p],
                                            psr, op=OP.mult)

                for h in range(NH):
                    ksl = sb.tile([P, KC, P], BF16, tag="kslab", bufs=2,
                                  name="ksl")
                    nc.sync.dma_start(
                        ksl[:, ds(0, n_slots)],
                        kts[:, h].rearrange("p (c x) -> p c x", x=P)[
                            :, ds(0, n_slots)],
                    )
                    vsl = sb.tile([P, KC, HD], BF16, tag="vslab", bufs=2,
                                  name="vsl")
                    nc.sync.dma_start_transpose(
                        vsl[:, ds(0, n_slots)],
                        vT[ds(h * HD, HD), ds(0, n_slots * P)],
                    )
                    psout = psum.tile([P, QT], F32, tag="mm", bufs=5,
                                      name="psout")
                    esum = sb.tile([P, QT], F32, tag="acc", bufs=3,
                                   name="esum")
                    esum_b = sb.tile([P, QT], F32, tag="sin_t", bufs=1,
                                     name="esum_b")
                    qsl = q_res[:, h, ds(qsl_off, QT)]

                    ets = {}

                    def emit_score(s):
                        pss = psum.tile([P, QT], F32, tag="mm", bufs=5,
                                        name="pss")
                        nc.tensor.matmul(pss, lhsT=ksl[:, s], rhs=qsl)
                        et = sb.tile([P, QT], BF16, tag="exp", bufs=4,
                                     name="et")
                        nc.scalar.activation(et, pss, AF.Exp,
                                             bias=biast[:, t, ds(s, 1)])
                        if s in mpos:
                            nc.vector.tensor_tensor(et, et, mt[:, mpos[s]],
                                                    op=OP.mult)
                        ets[s] = et

                    for s in range(min(LOOKAHEAD, n_slots)):
                        emit_score(s)
                    # previous head's normalization, pipelined behind our
                    # prologue so the PE never waits on its denominator
                    if pending is not None:
                        finish_norm(pending)
                    # previous tile's out-projection, one feature chunk per
                    # head: fills the PE while Scalar works on our exps
                    if prev_ot is not None:
                        outproj_chunk(prev_ot, qsl_off - QT, h)
                    for s in range(n_slots):
                        if s + LOOKAHEAD < n_slots:
                            emit_score(s + LOOKAHEAD)
                        et = ets.pop(s)
                        nc.tensor.matmul(
                            psout,
                            lhsT=vsl[:, s],
                            rhs=et,
                            start=(s == 0),
                            stop=(s == n_slots - 1),
                        )
                        # denominator accumulation off the PE: two chains
                        # partitioned in time — GpSimd takes the early
                        # slots, DVE the late ones (less SBUF-port overlap)
                        half = n_slots // 2
                        if s == 0:
                            nc.vector.tensor_copy(_r(esum_b), et)
                        elif s < half:
                            nc.gpsimd.tensor_tensor(_r(esum_b), esum_b, et,
                                                    op=OP.add)
                        elif s == half:
                            nc.vector.tensor_copy(_r(esum), et)
                        else:
                            nc.vector.tensor_tensor(_r(esum), esum, et,
                                                    op=OP.add)
                    nc.vector.tensor_tensor(_r(esum), esum, esum_b, op=OP.add)
                    pending = (psout, esum, h)
                finish_norm(pending)
                prev_ot = ot_res

            # ---- out-projection for the last q tile ----
            for e in range(NH):
                outproj_chunk(prev_ot, (NQ - 1) * QT, e)

    nc.compile()
    return nc


# --------------------------------------------------------------------------
# Host-side prep and driver
# --------------------------------------------------------------------------

_PERMS = {0: (0, 1, 2, 3), 1: (1, 0, 3, 2)}


def make_host_data(x, w_in, w_out, q_gamma, q_beta, k_gamma, k_beta):
    """Build per-core in_maps (list of dicts) + assembly metadata."""
    import ml_dtypes
    bf16 = ml_dtypes.bfloat16

    B = x.shape[0]
    n_cores = 2 * B

    w64 = np.asarray(w_in, np.float64)
    wq = w64[0:D]
    wk = w64[D:2 * D]
    wv = w64[2 * D:3 * D]
    wq_c = wq - wq.mean(axis=0, keepdims=True)
    wk_c = wk - wk.mean(axis=0, keepdims=True)
    wqkT2 = np.concatenate([wq_c.T, wk_c.T], axis=1)   # [D, 2D]
    wqk_t = np.ascontiguousarray(
        wqkT2.reshape(DC, P, 2 * NH, P).transpose(2, 1, 0, 3)
    ).astype(bf16)
    wvT = wv.T  # [D(d), D(f)]
    wv_t = np.ascontiguousarray(
        wvT.reshape(DC, P, NH, P)
    ).astype(bf16)
    woT = np.asarray(w_out, np.float64).T  # [D(hfeat), D(eout)]
    wo_t = np.ascontiguousarray(
        woT.reshape(NH, P, NH, P).transpose(2, 1, 0, 3)
    ).astype(bf16)

    inv = 1.0 / (10000.0 ** (np.arange(0, HD, 2, dtype=np.float64) / HD))
    tpos = np.arange(S, dtype=np.float64)
    fr = np.outer(tpos, inv)
    emb = np.concatenate([fr, fr], axis=-1)  # [S, HD]
    cosT = np.cos(emb).T  # [HD, S]
    sinT = np.sin(emb).T

    h2 = HD // 2
    rotmT = np.zeros((P, P), np.float32)
    for p in range(h2):
        rotmT[p + h2, p] = -1.0
    for p in range(h2, HD):
        rotmT[p - h2, p] = 1.0
    rotm = rotmT.astype(bf16)

    scale = 1.0 / math.sqrt(HD)
    gq_a = np.ascontiguousarray(
        (np.asarray(q_gamma, np.float64) * scale).reshape(NH, P).T
    ).astype(np.float32)
    bq_a = np.ascontiguousarray(
        (np.asarray(q_beta, np.float64) * scale).reshape(NH, P).T
    ).astype(np.float32)
    gk_a = np.ascontiguousarray(
        np.asarray(k_gamma, np.float32).reshape(NH, P).T
    )
    bk_a = np.ascontiguousarray(
        np.asarray(k_beta, np.float32).reshape(NH, P).T
    )
    onesc = np.ones((P, 1), np.float32)
    onesr = np.ones((1, P), np.float32)

    xb_T = {}
    in_maps = []
    meta = []
    for c in range(n_cores):
        b = c // 2
        r = c % 2
        perm = _PERMS[r]
        ptok = np.concatenate(
            [np.arange(pb * QT, (pb + 1) * QT) for pb in perm]
        )
        if b not in xb_T:
            xb_T[b] = np.ascontiguousarray(
                np.asarray(x[b], np.float32).T
            )  # [D, S] f32
        xT = np.ascontiguousarray(xb_T[b][:, ptok]).astype(bf16)
        cosp = np.ascontiguousarray(cosT[:, ptok]).astype(bf16)
        sinp = np.ascontiguousarray(sinT[:, ptok]).astype(bf16)

        # masks in PERMUTED kv space; q slabs at permuted positions Q_POS.
        # Elementwise masks only on diagonal slots; other slots use the
        # per-row exp bias: -EXP_BIAS for fully valid rows, BIAS_INVALID
        # for fully invalid rows.
        masks = np.zeros([NQ, P, MAXM, QT], np.float32)
        biast = np.full([P, NQ, 16], -EXP_BIAS, np.float32)
        for t in range(NQ):
            gq_tok = ptok[Q_POS[t] * QT + np.arange(QT)]
            gq_max = gq_tok.max()
            for mi, kc in enumerate(MASKED[t]):
                gkv = ptok[kc * P + np.arange(P)]
                masks[t, :, mi, :] = (
                    gkv[:, None] <= gq_tok[None, :]
                ).astype(np.float32)
            for kc in range(16):
                gkv = ptok[kc * P + np.arange(P)]
                biast[:, t, kc] = np.where(gkv <= gq_max, -EXP_BIAS,
                                           BIAS_INVALID)
        masks = masks.astype(bf16)

        qtok = np.concatenate(
            [np.arange(perm[pq] * QT, (perm[pq] + 1) * QT) for pq in Q_POS]
        )
        in_maps.append(dict(
            xT=xT, wqk=wqk_t, wv=wv_t, wo=wo_t,
            cos=cosp, sin=sinp,
            gq=gq_a, bq=bq_a, gk=gk_a, bk=bk_a, masks=masks,
            biast=biast, onesc=onesc, onesr=onesr, rotm=rotm,
        ))
        meta.append(dict(b=b, qtok=qtok))
    return in_maps, meta


_PROGRAM_CACHE = {}


def _get_program():
    if "full" not in _PROGRAM_CACHE:
        _PROGRAM_CACHE["full"] = build_program()
    return _PROGRAM_CACHE["full"]


def run_full(x, w_in, w_out, q_gamma, q_beta, k_gamma, k_beta,
             trace=False):
    from concourse.bass_utils import run_bass_kernel_spmd

    B = x.shape[0]
    n_cores = 2 * B
    in_maps, meta = make_host_data(
        x, w_in, w_out, q_gamma, q_beta, k_gamma, k_beta,
    )
    nc = _get_program()
    res = run_bass_kernel_spmd(
        nc, in_maps, core_ids=list(range(n_cores)), trace=trace,
    )
    out = np.empty((B, S, D), np.float32)
    for c in range(n_cores):
        o = res.results[c]["out"]  # [D, NQTOK]
        out[meta[c]["b"], meta[c]["qtok"], :] = o.T
    return out, res


def kernel(x, w_in, w_out, q_gamma, q_beta, k_gamma, k_beta, n_heads=16,
           **_ignored):
    x = np.asarray(x, np.float32)
    assert int(np.asarray(n_heads)) * HD == x.shape[-1]
    out, _ = run_full(
        np.asarray(x, np.float32),
        np.asarray(w_in, np.float32),
        np.asarray(w_out, np.float32),
        np.asarray(q_gamma, np.float32),
        np.asarray(q_beta, np.float32),
        np.asarray(k_gamma, np.float32),
        np.asarray(k_beta, np.float32),
    )
    return out


# revision 36
# speedup vs baseline: 1.0228x; 1.0228x over previous
"""Trainium2 Bass kernel for a custom attention block (qkv-proj + LN(q,k) +
RoPE + causal attention + out-proj), distributed over 8 NeuronCores.

Sharding: 2 cores per batch (B=4). Core role r=c%2 takes q-token blocks
{0,3} (r=0) or {1,2} (r=1) of 512 tokens; every core computes K/V for the
full 2048-token sequence of its batch (no collectives). The compiled
program is identical on all cores; per-core differences are input data
only. To keep the q-slab offsets compile-time-constant, each core sees
the sequence in a per-role BLOCK PERMUTATION (r=0: 0,1,2,3; r=1:
1,0,3,2), so its q blocks always sit at permuted positions {0,3} and the
causal diagonal lands on the same slot indices for both roles. The
cos/sin tables, causal masks, exp row-biases and output assembly are
permutation-aware host data.

All matmuls run in bf16 (same PE rate as fp32r but faster weight loads,
half the DMA/SBUF), with fp32 PSUM accumulation. x is SBUF-resident; q
stays SBUF-resident post-rope; k round-trips through DRAM feature-major;
v is computed feature-major (so the PE reuses each stationary weight
tile across 4 moving tiles) and transposed to token-major on the fly by
DMA-transpose loads during attention.

Engine split: PE does projections/scores/PV; Scalar does exp, squares
and PSUM->SBUF copies; DVE does LN/rope muls, masks and half the
softmax-denominator accumulation; GpSimd does the other half plus the
rope add and LN sumsq chains. Softmax denominator = chained elementwise
adds of the exp tiles + one ones-matmul partition reduction per head.
"""

import math

import numpy as np

import concourse.bass as bass
import concourse.mybir as mybir
import concourse.tile as tile
from concourse import bacc
from concourse.bass import ds

F32 = mybir.dt.float32
F32R = mybir.dt.float32r
BF16 = mybir.dt.bfloat16
AF = mybir.ActivationFunctionType
OP = mybir.AluOpType

P = 128
HD = 128
D = 2048
S = 2048
NH = D // HD          # 16 heads = feature chunks
DC = D // P           # 16 contraction chunks
NQTOK = 1024          # q tokens per core
QT = 512              # q/attention tile width (moving dim)
NQ = NQTOK // QT      # 2 q tiles per core
EXP_BIAS = 8.0
EPS = 1e-5
SLOTS = (8, 16)       # kv 128-chunks per q tile (max over the two roles)
# elementwise masks only on the diagonal slots (identical for both roles
# thanks to the block permutation); everything else is handled by the
# per-row exp bias (-EXP_BIAS valid / BIAS_INVALID invalid).
MASKED = ((0, 1, 2, 3), (12, 13, 14, 15))
MAXM = 4
BIAS_INVALID = -30.0
Q_POS = (0, 3)        # structural (permuted) block positions of q slabs
KC = S // P           # 16 kv chunks
LOOKAHEAD = 2         # attention score-slot software pipeline depth


def _r(ap):
    """fp32 -> fp32r view for matmul operands."""
    return ap.bitcast(F32R)


def _v3(ap):
    """[P, n*128] AP -> [P, n, 128] view (avoids 1-free-dim DMA splits)."""
    return ap.rearrange("p (a x) -> p a x", x=P)


def build_program():
    nc = bacc.Bacc("TRN2", target_bir_lowering=False, debug=False)

    # ---- I/O ----
    xT_i = nc.dram_tensor("xT", [D, S], BF16, kind="ExternalInput").ap()
    wqk_i = nc.dram_tensor("wqk", [2 * NH, P, DC, P], BF16,
                           kind="ExternalInput").ap()
    wv_i = nc.dram_tensor("wv", [DC, P, NH, P], BF16,
                          kind="ExternalInput").ap()
    wo_i = nc.dram_tensor("wo", [NH, P, NH, P], BF16,
                          kind="ExternalInput").ap()
    cos_i = nc.dram_tensor("cos", [HD, S], BF16, kind="ExternalInput").ap()
    sin_i = nc.dram_tensor("sin", [HD, S], BF16, kind="ExternalInput").ap()
    gq_i = nc.dram_tensor("gq", [P, NH], F32, kind="ExternalInput").ap()
    bq_i = nc.dram_tensor("bq", [P, NH], F32, kind="ExternalInput").ap()
    gk_i = nc.dram_tensor("gk", [P, NH], F32, kind="ExternalInput").ap()
    bk_i = nc.dram_tensor("bk", [P, NH], F32, kind="ExternalInput").ap()
    masks_i = nc.dram_tensor("masks", [NQ, P, MAXM, QT], BF16,
                             kind="ExternalInput").ap()
    biast_i = nc.dram_tensor("biast", [P, NQ, 16], F32,
                             kind="ExternalInput").ap()
    onesc_i = nc.dram_tensor("onesc", [P, 1], F32, kind="ExternalInput").ap()
    onesr_i = nc.dram_tensor("onesr", [1, P], F32, kind="ExternalInput").ap()
    rotm_i = nc.dram_tensor("rotm", [P, P], BF16, kind="ExternalInput").ap()
    out_t = nc.dram_tensor("out", [D, NQTOK], F32, kind="ExternalOutput").ap()

    with tile.TileContext(nc) as tc:
        import contextlib

        ctx = contextlib.ExitStack()
        with ctx:
            sb = ctx.enter_context(tc.tile_pool(name="sb", bufs=1))
            psum = ctx.enter_context(tc.tile_pool(name="ps", bufs=1, space="PSUM"))
            dram = ctx.enter_context(tc.tile_pool(name="dram", bufs=1, space="DRAM"))

            # ---- DRAM scratch ----
            kts = dram.tile([P, NH, S], BF16, tag="kts", name="kts")
            vT = dram.tile([D, S], BF16, tag="vT", name="vT")

            # ---- constants / small inputs ----
            ones_col = sb.tile([P, 1], F32, tag="ones_col", name="ones_col")
            nc.sync.dma_start(_r(ones_col), _r(onesc_i))
            ones_row = sb.tile([1, P], F32, tag="ones_row", name="ones_row")
            nc.sync.dma_start(_r(ones_row), _r(onesr_i))
            eps1 = sb.tile([1, 1], F32, tag="eps1", name="eps1")
            nc.vector.memset(eps1, EPS)
            zero1 = sb.tile([1, 1], F32, tag="zero1", name="zero1")
            nc.vector.memset(zero1, 0.0)
            biast = sb.tile([P, NQ, 16], F32, tag="biast", name="biast")
            nc.sync.dma_start(biast, biast_i)
            rotm = sb.tile([P, P], BF16, tag="rotm", name="rotm")
            nc.sync.dma_start(rotm, rotm_i)
            gq = sb.tile([P, NH], F32, tag="gq", name="gq")
            nc.sync.dma_start(gq, gq_i)
            bq = sb.tile([P, NH], F32, tag="bq", name="bq")
            nc.sync.dma_start(bq, bq_i)
            gk = sb.tile([P, NH], F32, tag="gk", name="gk")
            nc.sync.dma_start(gk, gk_i)
            bk = sb.tile([P, NH], F32, tag="bk", name="bk")
            nc.sync.dma_start(bk, bk_i)
            cos_t = sb.tile([HD, S], BF16, tag="cos_t", name="cos_t")
            nc.sync.dma_start(_v3(cos_t), _v3(cos_i))
            sin_t = sb.tile([HD, S], BF16, tag="sin_t", name="sin_t")
            nc.sync.dma_start(_v3(sin_t), _v3(sin_i))

            # ---- resident x: [128, DC, S] bf16 (64KB/partition) ----
            xsb = sb.tile([P, DC, S], BF16, tag="xsb", bufs=1, name="xsb")
            for d in range(DC):
                nc.sync.dma_start(xsb[:, d], xT_i[ds(d * P, P), :])

            # ---- resident q (post-LN+rope): [128, NH, NQTOK] bf16 ----
            q_res = sb.tile([P, NH, NQTOK], BF16, tag="q_res", bufs=1,
                            name="q_res")

            def proj_group(ec_base, slab_offs, q_dst_offs, g_sb, b_sb):
                """Project x -> feature-partition [128, QT] tiles for each
                token slab in this group. Emits the matmul phase and
                returns a closure that emits the LN + rope tail (so the
                caller can interleave it into the next group's matmul
                stream and keep the PE dense).

                slab_offs: compile-time token offsets into the permuted
                sequence (index into x and cos/sin). If q_dst_offs is not
                None the result lands at q_res[:, :, q_dst_off]; else it
                DMAs to kts[:, :, slab_off].
                """
                n_s = len(slab_offs)
                holds = []
                for i in range(n_s):
                    if q_dst_offs is not None:
                        holds.append(q_res[:, :, ds(q_dst_offs[i], QT)])
                    else:
                        h = sb.tile([P, NH, QT], BF16, tag="khold", bufs=2,
                                    name="khold")
                        holds.append(h)
                sqsums = []
                for i in range(n_s):
                    sqsums.append(sb.tile([P, QT], F32, tag="acc", bufs=3,
                                          name="sqsum"))
                for ec in range(NH):
                    w = sb.tile([P, DC, P], BF16, tag="w", bufs=2, name="w")
                    nc.sync.dma_start(w, wqk_i[ec_base + ec])
                    pss = [psum.tile([P, QT], F32, tag="mm", bufs=5, name="ps")
                           for _ in range(n_s)]
                    for d in range(DC):
                        for i in range(n_s):
                            nc.tensor.matmul(
                                pss[i],
                                lhsT=w[:, d],
                                rhs=xsb[:, d, ds(slab_offs[i], QT)],
                                start=(d == 0),
                                stop=(d == DC - 1),
                            )
                    for i in range(n_s):
                        nc.scalar.copy(holds[i][:, ec], pss[i])
                        sq = sb.tile([P, QT], BF16, tag="sq", bufs=2,
                                     name="sq")
                        nc.scalar.square(sq, pss[i])
                        if ec == 0:
                            nc.vector.tensor_copy(_r(sqsums[i]), sq)
                        else:
                            nc.gpsimd.tensor_tensor(_r(sqsums[i]), sqsums[i],
                                                    sq, op=OP.add)

                def tail():
                    for i in range(n_s):
                        hold = holds[i]
                        csl = ds(slab_offs[i], QT)
                        # per-token sumsq: partition-sum via ones-matmul
                        pstat = psum.tile([1, QT], F32, tag="stat", bufs=3,
                                          name="pstat")
                        nc.tensor.matmul(pstat, lhsT=_r(ones_col),
                                         rhs=_r(sqsums[i]))
                        # rsig = exp(-0.5 * ln(sumsq/D + eps))
                        lnv = sb.tile([1, QT], F32, tag="stats_sb", bufs=4,
                                      name="lnv")
                        nc.scalar.activation(lnv, pstat, AF.Ln,
                                             scale=1.0 / D, bias=eps1)
                        rsig = sb.tile([1, QT], F32, tag="stats_sb", bufs=4,
                                       name="rsig")
                        nc.scalar.activation(_r(rsig), lnv, AF.Exp,
                                             bias=zero1, scale=-0.5)
                        ps_rep = psum.tile([P, QT], F32, tag="mm", bufs=5,
                                           name="ps_rep")
                        nc.tensor.matmul(ps_rep, lhsT=_r(ones_row),
                                         rhs=_r(rsig))
                        # pass 1: LN apply on all chunks (DVE)
                        for ec in range(NH):
                            ch = hold[:, ec]
                            nc.vector.tensor_tensor(ch, ch, ps_rep,
                                                    op=OP.mult)
                            nc.vector.tensor_scalar(
                                ch, ch,
                                scalar1=g_sb[:, ds(ec, 1)],
                                scalar2=b_sb[:, ds(ec, 1)],
                                op0=OP.mult, op1=OP.add,
                            )
                        # pass 2: rope; rotation matmuls stream back-to-back
                        for ec in range(NH):
                            ch = hold[:, ec]
                            ps_rot = psum.tile([P, QT], F32, tag="mm",
                                               bufs=5, name="ps_rot")
                            nc.tensor.matmul(ps_rot, lhsT=rotm, rhs=ch)
                            tmp = sb.tile([P, QT], BF16, tag="rtmp", bufs=2,
                                          name="rtmp")
                            nc.vector.tensor_tensor(tmp, ps_rot,
                                                    sin_t[:, csl], op=OP.mult)
                            nc.vector.tensor_tensor(ch, ch, cos_t[:, csl],
                                                    op=OP.mult)
                            nc.gpsimd.tensor_tensor(ch, ch, tmp, op=OP.add)
                        if q_dst_offs is None:
                            nc.sync.dma_start(
                                kts[:, :, ds(slab_offs[i], QT)], hold
                            )

                return tail

            def v_chunk(f):
                """Phase V chunk: w-stationary (reused across 4 moving
                tiles), writes v^T feature-major to DRAM."""
                wvf = sb.tile([P, DC, P], BF16, tag="w", bufs=2, name="wvf")
                nc.sync.dma_start(wvf, wv_i[:, :, f, :].rearrange(
                    "d p j -> p d j"))
                psv = [psum.tile([P, QT], F32, tag="mm", bufs=5, name="psv")
                       for _ in range(4)]
                for d in range(DC):
                    for ts in range(4):
                        nc.tensor.matmul(
                            psv[ts],
                            lhsT=wvf[:, d],
                            rhs=xsb[:, d, ds(ts * QT, QT)],
                            start=(d == 0),
                            stop=(d == DC - 1),
                        )
                for ts in range(4):
                    vtsb = sb.tile([P, QT], BF16, tag="vsb", bufs=3,
                                   name="vtsb")
                    nc.scalar.copy(vtsb, psv[ts])
                    nc.gpsimd.dma_start(
                        _v3(vT[ds(f * P, P), ds(ts * QT, QT)]), _v3(vtsb)
                    )

            # ---- Projections: each group's LN/rope tail is emitted
            # inside the NEXT group's matmul stream so the PE stays dense.
            tail_q = proj_group(0, [Q_POS[0] * QT, Q_POS[1] * QT], [0, QT],
                                gq, bq)
            pending_tail = tail_q
            for g in range(4):
                t_k = proj_group(NH, [g * QT], None, gk, bk)
                pending_tail()
                pending_tail = t_k
            v_chunk(0)
            pending_tail()
            for f in range(1, NH):
                v_chunk(f)

            # ---- Attention + out-projection per q tile ----
            def outproj_chunk(ot_src, qoff, e):
                """One out-projection feature chunk; pure PE work used to
                fill exp-latency gaps in the attention stream."""
                wot = sb.tile([P, NH, P], BF16, tag="wot", bufs=2,
                              name="wot")
                nc.sync.dma_start(wot, wo_i[e])
                psf = psum.tile([P, QT], F32, tag="stat", bufs=3,
                                name="psf")
                for h in range(NH):
                    nc.tensor.matmul(
                        psf,
                        lhsT=wot[:, h],
                        rhs=ot_src[:, h],
                        start=(h == 0),
                        stop=(h == NH - 1),
                    )
                fsb = sb.tile([P, QT], F32, tag="fsb", bufs=2,
                              name="fsb")
                nc.scalar.copy(fsb, psf)
                nc.sync.dma_start(
                    _v3(out_t[ds(e * P, P), ds(qoff, QT)]), _v3(fsb)
                )

            prev_ot = None
            for t in range(NQ):
                qsl_off = t * QT
                n_slots = SLOTS[t]
                # masks overlay the (now dead) cos buffer
                mt = sb.tile([P, MAXM, QT], BF16, tag="cos_t", bufs=1,
                             name="mt")
                nc.sync.dma_start(mt, masks_i[t])
                mpos = {kc: i for i, kc in enumerate(MASKED[t])}
                ot_res = sb.tile([P, NH, QT], BF16, tag="khold", bufs=2,
                                 name="ot_res")
                pending = None

                def finish_norm(pending):
                    psout_p, esum_p, h_p = pending
                    psden = psum.tile([1, QT], F32, tag="stat", bufs=3,
                                      name="psden")
                    nc.tensor.matmul(psden, lhsT=_r(ones_col), rhs=_r(esum_p))
                    rec0 = sb.tile([1, QT], F32, tag="stats_sb", bufs=4,
                                   name="rec0")
                    with nc.allow_low_precision(
                        reason="denominator reciprocal, 18 bits is plenty"
                    ):
                        nc.vector.reciprocal_approx_fast(rec0, psden)
                    rec = sb.tile([1, QT], F32, tag="stats_sb", bufs=4,
                                  name="rec")
                    nc.vector.tensor_copy(_r(rec), rec0)
                    psr = psum.tile([P, QT], F32, tag="mm", bufs=5,
                                    name="psr")
                    nc.tensor.matmul(psr, lhsT=_r(ones_row), rhs=_r(rec))
                    nc.vector.tensor_copy(ot_res[:, h_p], psout_p)
                    nc.vector.tensor_tensor(ot_res[:, h_p], ot_res[:, h_p],
                                            psr, op=OP.mult)

                for h in range(NH):
                    ksl = sb.tile([P, KC, P], BF16, tag="kslab", bufs=2,
                                  name="ksl")
                    nc.sync.dma_start(
                        ksl[:, ds(0, n_slots)],
                        kts[:, h].rearrange("p (c x) -> p c x", x=P)[
                            :, ds(0, n_slots)],
                    )
                    vsl = sb.tile([P, KC, HD], BF16, tag="vslab", bufs=2,
                                  name="vsl")
                    nc.sync.dma_start_transpose(
                        vsl[:, ds(0, n_slots)],
                        vT[ds(h * HD, HD), ds(0, n_slots * P)],
                    )
                    psout = psum.tile([P, QT], F32, tag="mm", bufs=5,
                                      name="psout")
                    esum = sb.tile([P, QT], F32, tag="acc", bufs=3,
                                   name="esum")
                    esum_b = sb.tile([P, QT], F32, tag="sin_t", bufs=1,
                                     name="esum_b")
                    qsl = q_res[:, h, ds(qsl_off, QT)]

                    ets = {}

                    def emit_score(s):
                        pss = psum.tile([P, QT], F32, tag="mm", bufs=5,
                                        name="pss")
                        nc.tensor.matmul(pss, lhsT=ksl[:, s], rhs=qsl)
                        et = sb.tile([P, QT], BF16, tag="exp", bufs=4,
                                     name="et")
                        nc.scalar.activation(et, pss, AF.Exp,
                                             bias=biast[:, t, ds(s, 1)])
                        if s in mpos:
                            nc.vector.tensor_tensor(et, et, mt[:, mpos[s]],
                                                    op=OP.mult)
                        ets[s] = et

                    # previous tile's out-projection, one feature chunk per
                    # head: fills the PE while Scalar works on our exps
                    if prev_ot is not None:
                        outproj_chunk(prev_ot, qsl_off - QT, h)
                    for s in range(min(LOOKAHEAD, n_slots)):
                        emit_score(s)
                    # previous head's normalization, pipelined behind our
                    # prologue so the PE never waits on its denominator
                    if pending is not None:
                        finish_norm(pending)
                    for s in range(n_slots):
                        if s + LOOKAHEAD < n_slots:
                            emit_score(s + LOOKAHEAD)
                        et = ets.pop(s)
                        nc.tensor.matmul(
                            psout,
                            lhsT=vsl[:, s],
                            rhs=et,
                            start=(s == 0),
                            stop=(s == n_slots - 1),
                        )
                        # denominator accumulation off the PE: two chains
                        # partitioned in time — GpSimd takes the early
                        # slots, DVE the late ones (less SBUF-port overlap)
                        half = n_slots // 2
                        if s == 0:
                            nc.vector.tensor_copy(_r(esum_b), et)
                        elif s < half:
                            nc.gpsimd.tensor_tensor(_r(esum_b), esum_b, et,
                                                    op=OP.add)
                        elif s == half:
                            nc.vector.tensor_copy(_r(esum), et)
                        else:
                            nc.vector.tensor_tensor(_r(esum), esum, et,
                                                    op=OP.add)
                    nc.vector.tensor_tensor(_r(esum), esum, esum_b, op=OP.add)
                    pending = (psout, esum, h)
                finish_norm(pending)
                prev_ot = ot_res

            # ---- out-projection for the last q tile ----
            for e in range(NH):
                outproj_chunk(prev_ot, (NQ - 1) * QT, e)

    nc.compile()
    return nc


# --------------------------------------------------------------------------
# Host-side prep and driver
# --------------------------------------------------------------------------

_PERMS = {0: (0, 1, 2, 3), 1: (1, 0, 3, 2)}


def make_host_data(x, w_in, w_out, q_gamma, q_beta, k_gamma, k_beta):
    """Build per-core in_maps (list of dicts) + assembly metadata."""
    import ml_dtypes
    bf16 = ml_dtypes.bfloat16

    B = x.shape[0]
    n_cores = 2 * B

    w64 = np.asarray(w_in, np.float64)
    wq = w64[0:D]
    wk = w64[D:2 * D]
    wv = w64[2 * D:3 * D]
    wq_c = wq - wq.mean(axis=0, keepdims=True)
    wk_c = wk - wk.mean(axis=0, keepdims=True)
    wqkT2 = np.concatenate([wq_c.T, wk_c.T], axis=1)   # [D, 2D]
    wqk_t = np.ascontiguousarray(
        wqkT2.reshape(DC, P, 2 * NH, P).transpose(2, 1, 0, 3)
    ).astype(bf16)
    wvT = wv.T  # [D(d), D(f)]
    wv_t = np.ascontiguousarray(
        wvT.reshape(DC, P, NH, P)
    ).astype(bf16)
    woT = np.asarray(w_out, np.float64).T  # [D(hfeat), D(eout)]
    wo_t = np.ascontiguousarray(
        woT.reshape(NH, P, NH, P).transpose(2, 1, 0, 3)
    ).astype(bf16)

    inv = 1.0 / (10000.0 ** (np.arange(0, HD, 2, dtype=np.float64) / HD))
    tpos = np.arange(S, dtype=np.float64)
    fr = np.outer(tpos, inv)
    emb = np.concatenate([fr, fr], axis=-1)  # [S, HD]
    cosT = np.cos(emb).T  # [HD, S]
    sinT = np.sin(emb).T

    h2 = HD // 2
    rotmT = np.zeros((P, P), np.float32)
    for p in range(h2):
        rotmT[p + h2, p] = -1.0
    for p in range(h2, HD):
        rotmT[p - h2, p] = 1.0
    rotm = rotmT.astype(bf16)

    scale = 1.0 / math.sqrt(HD)
    gq_a = np.ascontiguousarray(
        (np.asarray(q_gamma, np.float64) * scale).reshape(NH, P).T
    ).astype(np.float32)
    bq_a = np.ascontiguousarray(
        (np.asarray(q_beta, np.float64) * scale).reshape(NH, P).T
    ).astype(np.float32)
    gk_a = np.ascontiguousarray(
        np.asarray(k_gamma, np.float32).reshape(NH, P).T
    )
    bk_a = np.ascontiguousarray(
        np.asarray(k_beta, np.float32).reshape(NH, P).T
    )
    onesc = np.ones((P, 1), np.float32)
    onesr = np.ones((1, P), np.float32)

    xb_T = {}
    in_maps = []
    meta = []
    for c in range(n_cores):
        b = c // 2
        r = c % 2
        perm = _PERMS[r]
        ptok = np.concatenate(
            [np.arange(pb * QT, (pb + 1) * QT) for pb in perm]
        )
        if b not in xb_T:
            xb_T[b] = np.ascontiguousarray(
                np.asarray(x[b], np.float32).T
            )  # [D, S] f32
        xT = np.ascontiguousarray(xb_T[b][:, ptok]).astype(bf16)
        cosp = np.ascontiguousarray(cosT[:, ptok]).astype(bf16)
        sinp = np.ascontiguousarray(sinT[:, ptok]).astype(bf16)

        # masks in PERMUTED kv space; q slabs at permuted positions Q_POS.
        # Elementwise masks only on diagonal slots; other slots use the
        # per-row exp bias: -EXP_BIAS for fully valid rows, BIAS_INVALID
        # for fully invalid rows.
        masks = np.zeros([NQ, P, MAXM, QT], np.float32)
        biast = np.full([P, NQ, 16], -EXP_BIAS, np.float32)
        for t in range(NQ):
            gq_tok = ptok[Q_POS[t] * QT + np.arange(QT)]
            gq_max = gq_tok.max()
            for mi, kc in enumerate(MASKED[t]):
                gkv = ptok[kc * P + np.arange(P)]
                masks[t, :, mi, :] = (
                    gkv[:, None] <= gq_tok[None, :]
                ).astype(np.float32)
            for kc in range(16):
                gkv = ptok[kc * P + np.arange(P)]
                biast[:, t, kc] = np.where(gkv <= gq_max, -EXP_BIAS,
                                           BIAS_INVALID)
        masks = masks.astype(bf16)

        qtok = np.concatenate(
            [np.arange(perm[pq] * QT, (perm[pq] + 1) * QT) for pq in Q_POS]
        )
        in_maps.append(dict(
            xT=xT, wqk=wqk_t, wv=wv_t, wo=wo_t,
            cos=cosp, sin=sinp,
            gq=gq_a, bq=bq_a, gk=gk_a, bk=bk_a, masks=masks,
            biast=biast, onesc=onesc, onesr=onesr, rotm=rotm,
        ))
        meta.append(dict(b=b, qtok=qtok))
    return in_maps, meta


_PROGRAM_CACHE = {}


def _get_program():
    if "full" not in _PROGRAM_CACHE:
        _PROGRAM_CACHE["full"] = build_program()
    return _PROGRAM_CACHE["full"]


def run_full(x, w_in, w_out, q_gamma, q_beta, k_gamma, k_beta,
             trace=False):
    from concourse.bass_utils import run_bass_kernel_spmd

    B = x.shape[0]
    n_cores = 2 * B
    in_maps, meta = make_host_data(
        x, w_in, w_out, q_gamma, q_beta, k_gamma, k_beta,
    )
    nc = _get_program()
    res = run_bass_kernel_spmd(
        nc, in_maps, core_ids=list(range(n_cores)), trace=trace,
    )
    out = np.empty((B, S, D), np.float32)
    for c in range(n_cores):
        o = res.results[c]["out"]  # [D, NQTOK]
        out[meta[c]["b"], meta[c]["qtok"], :] = o.T
    return out, res


def kernel(x, w_in, w_out, q_gamma, q_beta, k_gamma, k_beta, n_heads=16,
           **_ignored):
    x = np.asarray(x, np.float32)
    assert int(np.asarray(n_heads)) * HD == x.shape[-1]
    out, _ = run_full(
        np.asarray(x, np.float32),
        np.asarray(w_in, np.float32),
        np.asarray(w_out, np.float32),
        np.asarray(q_gamma, np.float32),
        np.asarray(q_beta, np.float32),
        np.asarray(k_gamma, np.float32),
        np.asarray(k_beta, np.float32),
    )
    return out


# revision 39
# speedup vs baseline: 1.0978x; 1.0733x over previous
"""Trainium2 Bass kernel for a custom attention block (qkv-proj + LN(q,k) +
RoPE + causal attention + out-proj), distributed over 8 NeuronCores.

Sharding: 2 cores per batch (B=4). Core role r=c%2 takes q-token blocks
{0,3} (r=0) or {1,2} (r=1) of 512 tokens; every core computes K/V for the
full 2048-token sequence of its batch (no collectives). The compiled
program is identical on all cores; per-core differences are input data
only. To keep the q-slab offsets compile-time-constant, each core sees
the sequence in a per-role BLOCK PERMUTATION (r=0: 0,1,2,3; r=1:
1,0,3,2), so its q blocks always sit at permuted positions {0,3} and the
causal diagonal lands on the same slot indices for both roles. The
cos/sin tables, causal masks, exp row-biases and output assembly are
permutation-aware host data.

All matmuls run in bf16 (same PE rate as fp32r but faster weight loads,
half the DMA/SBUF), with fp32 PSUM accumulation. x is SBUF-resident; q
stays SBUF-resident post-rope; k round-trips through DRAM feature-major;
v is computed feature-major (so the PE reuses each stationary weight
tile across 4 moving tiles) and transposed to token-major on the fly by
DMA-transpose loads during attention.

Engine split: PE does projections/scores/PV; Scalar does exp, squares
and PSUM->SBUF copies; DVE does LN/rope muls, masks and half the
softmax-denominator accumulation; GpSimd does the other half plus the
rope add and LN sumsq chains. Softmax denominator = chained elementwise
adds of the exp tiles + one ones-matmul partition reduction per head.
"""

import math

import numpy as np

import concourse.bass as bass
import concourse.mybir as mybir
import concourse.tile as tile
from concourse import bacc
from concourse.bass import ds

F32 = mybir.dt.float32
F32R = mybir.dt.float32r
BF16 = mybir.dt.bfloat16
AF = mybir.ActivationFunctionType
OP = mybir.AluOpType

P = 128
HD = 128
D = 2048
S = 2048
NH = D // HD          # 16 heads = feature chunks
DC = D // P           # 16 contraction chunks
NQTOK = 1024          # q tokens per core
QT = 512              # q/attention tile width (moving dim)
NQ = NQTOK // QT      # 2 q tiles per core
EXP_BIAS = 8.0
EPS = 1e-5
SLOTS = (8, 16)       # kv 128-chunks per q tile (max over the two roles)
# elementwise masks only on the diagonal slots (identical for both roles
# thanks to the block permutation); everything else is handled by the
# per-row exp bias (-EXP_BIAS valid / BIAS_INVALID invalid).
MASKED = ((0, 1, 2, 3), (12, 13, 14, 15))
MAXM = 4
BIAS_INVALID = -30.0
Q_POS = (0, 3)        # structural (permuted) block positions of q slabs
KC = S // P           # 16 kv chunks
LOOKAHEAD = 3         # attention score-slot software pipeline depth


def _r(ap):
    """fp32 -> fp32r view for matmul operands."""
    return ap.bitcast(F32R)


def _v3(ap):
    """[P, n*128] AP -> [P, n, 128] view (avoids 1-free-dim DMA splits)."""
    return ap.rearrange("p (a x) -> p a x", x=P)


def build_program():
    nc = bacc.Bacc("TRN2", target_bir_lowering=False, debug=False)

    # ---- I/O ----
    xT_i = nc.dram_tensor("xT", [D, S], BF16, kind="ExternalInput").ap()
    wqk_i = nc.dram_tensor("wqk", [2 * NH, P, DC, P], BF16,
                           kind="ExternalInput").ap()
    wv_i = nc.dram_tensor("wv", [DC, P, NH, P], BF16,
                          kind="ExternalInput").ap()
    wo_i = nc.dram_tensor("wo", [NH, P, NH, P], BF16,
                          kind="ExternalInput").ap()
    cos_i = nc.dram_tensor("cos", [HD, S], BF16, kind="ExternalInput").ap()
    sin_i = nc.dram_tensor("sin", [HD, S], BF16, kind="ExternalInput").ap()
    gq_i = nc.dram_tensor("gq", [P, NH], F32, kind="ExternalInput").ap()
    bq_i = nc.dram_tensor("bq", [P, NH], F32, kind="ExternalInput").ap()
    gk_i = nc.dram_tensor("gk", [P, NH], F32, kind="ExternalInput").ap()
    bk_i = nc.dram_tensor("bk", [P, NH], F32, kind="ExternalInput").ap()
    masks_i = nc.dram_tensor("masks", [NQ, P, MAXM, QT], BF16,
                             kind="ExternalInput").ap()
    biast_i = nc.dram_tensor("biast", [P, NQ, 16], F32,
                             kind="ExternalInput").ap()
    onesc_i = nc.dram_tensor("onesc", [P, 1], F32, kind="ExternalInput").ap()
    onesr_i = nc.dram_tensor("onesr", [1, P], F32, kind="ExternalInput").ap()
    rotm_i = nc.dram_tensor("rotm", [P, P], BF16, kind="ExternalInput").ap()
    out_t = nc.dram_tensor("out", [D, NQTOK], F32, kind="ExternalOutput").ap()

    with tile.TileContext(nc) as tc:
        import contextlib

        ctx = contextlib.ExitStack()
        with ctx:
            sb = ctx.enter_context(tc.tile_pool(name="sb", bufs=1))
            psum = ctx.enter_context(tc.tile_pool(name="ps", bufs=1, space="PSUM"))
            dram = ctx.enter_context(tc.tile_pool(name="dram", bufs=1, space="DRAM"))

            # ---- DRAM scratch ----
            kts = dram.tile([P, NH, S], BF16, tag="kts", name="kts")
            vT = dram.tile([D, S], BF16, tag="vT", name="vT")

            # ---- constants / small inputs ----
            ones_col = sb.tile([P, 1], F32, tag="ones_col", name="ones_col")
            nc.sync.dma_start(_r(ones_col), _r(onesc_i))
            ones_row = sb.tile([1, P], F32, tag="ones_row", name="ones_row")
            nc.sync.dma_start(_r(ones_row), _r(onesr_i))
            eps1 = sb.tile([1, 1], F32, tag="eps1", name="eps1")
            nc.vector.memset(eps1, EPS)
            zero1 = sb.tile([1, 1], F32, tag="zero1", name="zero1")
            nc.vector.memset(zero1, 0.0)
            biast = sb.tile([P, NQ, 16], F32, tag="biast", name="biast")
            nc.sync.dma_start(biast, biast_i)
            rotm = sb.tile([P, P], BF16, tag="rotm", name="rotm")
            nc.sync.dma_start(rotm, rotm_i)
            gq = sb.tile([P, NH], F32, tag="gq", name="gq")
            nc.sync.dma_start(gq, gq_i)
            bq = sb.tile([P, NH], F32, tag="bq", name="bq")
            nc.sync.dma_start(bq, bq_i)
            gk = sb.tile([P, NH], F32, tag="gk", name="gk")
            nc.sync.dma_start(gk, gk_i)
            bk = sb.tile([P, NH], F32, tag="bk", name="bk")
            nc.sync.dma_start(bk, bk_i)
            cos_t = sb.tile([HD, S], BF16, tag="cos_t", name="cos_t")
            nc.sync.dma_start(_v3(cos_t), _v3(cos_i))
            sin_t = sb.tile([HD, S], BF16, tag="sin_t", name="sin_t")
            nc.sync.dma_start(_v3(sin_t), _v3(sin_i))

            # ---- resident x: [128, DC, S] bf16 (64KB/partition) ----
            xsb = sb.tile([P, DC, S], BF16, tag="xsb", bufs=1, name="xsb")
            for d in range(DC):
                nc.sync.dma_start(xsb[:, d], xT_i[ds(d * P, P), :])

            # ---- resident q (post-LN+rope): [128, NH, NQTOK] bf16 ----
            q_res = sb.tile([P, NH, NQTOK], BF16, tag="q_res", bufs=1,
                            name="q_res")

            def proj_group(ec_base, slab_offs, q_dst_offs, g_sb, b_sb):
                """Project x -> feature-partition [128, QT] tiles for each
                token slab in this group. Emits the matmul phase and
                returns a closure that emits the LN + rope tail (so the
                caller can interleave it into the next group's matmul
                stream and keep the PE dense).

                slab_offs: compile-time token offsets into the permuted
                sequence (index into x and cos/sin). If q_dst_offs is not
                None the result lands at q_res[:, :, q_dst_off]; else it
                DMAs to kts[:, :, slab_off].
                """
                n_s = len(slab_offs)
                holds = []
                for i in range(n_s):
                    if q_dst_offs is not None:
                        holds.append(q_res[:, :, ds(q_dst_offs[i], QT)])
                    else:
                        h = sb.tile([P, NH, QT], BF16, tag="khold", bufs=2,
                                    name="khold")
                        holds.append(h)
                sqsums = []
                for i in range(n_s):
                    sqsums.append(sb.tile([P, QT], F32, tag="acc", bufs=3,
                                          name="sqsum"))
                for ec in range(NH):
                    w = sb.tile([P, DC, P], BF16, tag="w", bufs=2, name="w")
                    nc.sync.dma_start(w, wqk_i[ec_base + ec])
                    pss = [psum.tile([P, QT], F32, tag="mm", bufs=5, name="ps")
                           for _ in range(n_s)]
                    for d in range(DC):
                        for i in range(n_s):
                            nc.tensor.matmul(
                                pss[i],
                                lhsT=w[:, d],
                                rhs=xsb[:, d, ds(slab_offs[i], QT)],
                                start=(d == 0),
                                stop=(d == DC - 1),
                            )
                    for i in range(n_s):
                        nc.scalar.copy(holds[i][:, ec], pss[i])
                        sq = sb.tile([P, QT], BF16, tag="sq", bufs=2,
                                     name="sq")
                        nc.scalar.square(sq, pss[i])
                        if ec == 0:
                            nc.vector.tensor_copy(_r(sqsums[i]), sq)
                        else:
                            nc.gpsimd.tensor_tensor(_r(sqsums[i]), sqsums[i],
                                                    sq, op=OP.add)

                def tail():
                    for i in range(n_s):
                        hold = holds[i]
                        csl = ds(slab_offs[i], QT)
                        # per-token sumsq: partition-sum via ones-matmul
                        pstat = psum.tile([1, QT], F32, tag="stat", bufs=3,
                                          name="pstat")
                        nc.tensor.matmul(pstat, lhsT=_r(ones_col),
                                         rhs=_r(sqsums[i]))
                        # rsig = exp(-0.5 * ln(sumsq/D + eps))
                        lnv = sb.tile([1, QT], F32, tag="stats_sb", bufs=4,
                                      name="lnv")
                        nc.scalar.activation(lnv, pstat, AF.Ln,
                                             scale=1.0 / D, bias=eps1)
                        rsig = sb.tile([1, QT], F32, tag="stats_sb", bufs=4,
                                       name="rsig")
                        nc.scalar.activation(_r(rsig), lnv, AF.Exp,
                                             bias=zero1, scale=-0.5)
                        ps_rep = psum.tile([P, QT], F32, tag="mm", bufs=5,
                                           name="ps_rep")
                        nc.tensor.matmul(ps_rep, lhsT=_r(ones_row),
                                         rhs=_r(rsig))
                        # pass 1: LN apply on all chunks (DVE)
                        for ec in range(NH):
                            ch = hold[:, ec]
                            nc.vector.tensor_tensor(ch, ch, ps_rep,
                                                    op=OP.mult)
                            nc.vector.tensor_scalar(
                                ch, ch,
                                scalar1=g_sb[:, ds(ec, 1)],
                                scalar2=b_sb[:, ds(ec, 1)],
                                op0=OP.mult, op1=OP.add,
                            )
                        # pass 2: rope; rotation matmuls stream back-to-back
                        for ec in range(NH):
                            ch = hold[:, ec]
                            ps_rot = psum.tile([P, QT], F32, tag="mm",
                                               bufs=5, name="ps_rot")
                            nc.tensor.matmul(ps_rot, lhsT=rotm, rhs=ch)
                            tmp = sb.tile([P, QT], BF16, tag="rtmp", bufs=2,
                                          name="rtmp")
                            nc.vector.tensor_tensor(tmp, ps_rot,
                                                    sin_t[:, csl], op=OP.mult)
                            nc.vector.tensor_tensor(ch, ch, cos_t[:, csl],
                                                    op=OP.mult)
                            nc.gpsimd.tensor_tensor(ch, ch, tmp, op=OP.add)
                        if q_dst_offs is None:
                            nc.sync.dma_start(
                                kts[:, :, ds(slab_offs[i], QT)], hold
                            )

                return tail

            def v_chunk(f):
                """Phase V chunk: w-stationary (reused across 4 moving
                tiles), writes v^T feature-major to DRAM."""
                wvf = sb.tile([P, DC, P], BF16, tag="w", bufs=2, name="wvf")
                nc.sync.dma_start(wvf, wv_i[:, :, f, :].rearrange(
                    "d p j -> p d j"))
                psv = [psum.tile([P, QT], F32, tag="mm", bufs=5, name="psv")
                       for _ in range(4)]
                for d in range(DC):
                    for ts in range(4):
                        nc.tensor.matmul(
                            psv[ts],
                            lhsT=wvf[:, d],
                            rhs=xsb[:, d, ds(ts * QT, QT)],
                            start=(d == 0),
                            stop=(d == DC - 1),
                        )
                for ts in range(4):
                    vtsb = sb.tile([P, QT], BF16, tag="vsb", bufs=3,
                                   name="vtsb")
                    nc.scalar.copy(vtsb, psv[ts])
                    nc.gpsimd.dma_start(
                        _v3(vT[ds(f * P, P), ds(ts * QT, QT)]), _v3(vtsb)
                    )

            # ---- Projections: each group's LN/rope tail is emitted
            # inside the NEXT group's matmul stream so the PE stays dense.
            pending_tail = proj_group(0, [Q_POS[0] * QT], [0], gq, bq)
            t_q1 = proj_group(0, [Q_POS[1] * QT], [QT], gq, bq)
            pending_tail()
            pending_tail = t_q1
            for g in range(4):
                t_k = proj_group(NH, [g * QT], None, gk, bk)
                pending_tail()
                pending_tail = t_k
            v_chunk(0)
            pending_tail()
            for f in range(1, NH):
                v_chunk(f)

            # ---- Attention + out-projection per q tile ----
            def outproj_chunk(ot_src, qoff, e):
                """One out-projection feature chunk; pure PE work used to
                fill exp-latency gaps in the attention stream."""
                wot = sb.tile([P, NH, P], BF16, tag="wot", bufs=2,
                              name="wot")
                nc.sync.dma_start(wot, wo_i[e])
                psf = psum.tile([P, QT], F32, tag="stat", bufs=3,
                                name="psf")
                for h in range(NH):
                    nc.tensor.matmul(
                        psf,
                        lhsT=wot[:, h],
                        rhs=ot_src[:, h],
                        start=(h == 0),
                        stop=(h == NH - 1),
                    )
                fsb = sb.tile([P, QT], F32, tag="fsb", bufs=2,
                              name="fsb")
                nc.scalar.copy(fsb, psf)
                nc.sync.dma_start(
                    _v3(out_t[ds(e * P, P), ds(qoff, QT)]), _v3(fsb)
                )

            prev_ot = None
            for t in range(NQ):
                qsl_off = t * QT
                n_slots = SLOTS[t]
                # masks overlay the (now dead) cos buffer
                mt = sb.tile([P, MAXM, QT], BF16, tag="cos_t", bufs=1,
                             name="mt")
                nc.sync.dma_start(mt, masks_i[t])
                mpos = {kc: i for i, kc in enumerate(MASKED[t])}
                ot_res = sb.tile([P, NH, QT], BF16, tag="khold", bufs=2,
                                 name="ot_res")
                pending = None

                def finish_norm(pending):
                    psout_p, esum_p, h_p = pending
                    psden = psum.tile([1, QT], F32, tag="stat", bufs=3,
                                      name="psden")
                    nc.tensor.matmul(psden, lhsT=_r(ones_col), rhs=_r(esum_p))
                    rec0 = sb.tile([1, QT], F32, tag="stats_sb", bufs=4,
                                   name="rec0")
                    with nc.allow_low_precision(
                        reason="denominator reciprocal, 18 bits is plenty"
                    ):
                        nc.vector.reciprocal_approx_fast(rec0, psden)
                    rec = sb.tile([1, QT], F32, tag="stats_sb", bufs=4,
                                  name="rec")
                    nc.vector.tensor_copy(_r(rec), rec0)
                    psr = psum.tile([P, QT], F32, tag="stat", bufs=3,
                                    name="psr")
                    nc.tensor.matmul(psr, lhsT=_r(ones_row), rhs=_r(rec))
                    nc.vector.tensor_copy(ot_res[:, h_p], psout_p)
                    nc.vector.tensor_tensor(ot_res[:, h_p], ot_res[:, h_p],
                                            psr, op=OP.mult)

                for h in range(NH):
                    ksl = sb.tile([P, KC, P], BF16, tag="kslab", bufs=2,
                                  name="ksl")
                    nc.sync.dma_start(
                        ksl[:, ds(0, n_slots)],
                        kts[:, h].rearrange("p (c x) -> p c x", x=P)[
                            :, ds(0, n_slots)],
                    )
                    vsl = sb.tile([P, KC, HD], BF16, tag="vslab", bufs=2,
                                  name="vsl")
                    nc.sync.dma_start_transpose(
                        vsl[:, ds(0, n_slots)],
                        vT[ds(h * HD, HD), ds(0, n_slots * P)],
                    )
                    psout = psum.tile([P, QT], F32, tag="mm", bufs=5,
                                      name="psout")
                    esum = sb.tile([P, QT], F32, tag="acc", bufs=3,
                                   name="esum")
                    esum_b = sb.tile([P, QT], F32, tag="sin_t", bufs=1,
                                     name="esum_b")
                    qsl = q_res[:, h, ds(qsl_off, QT)]

                    ets = {}

                    def emit_score(s):
                        pss = psum.tile([P, QT], F32, tag="mm", bufs=5,
                                        name="pss")
                        nc.tensor.matmul(pss, lhsT=ksl[:, s], rhs=qsl)
                        et = sb.tile([P, QT], BF16, tag="exp", bufs=4,
                                     name="et")
                        nc.scalar.activation(et, pss, AF.Exp,
                                             bias=biast[:, t, ds(s, 1)])
                        if s in mpos:
                            nc.vector.tensor_tensor(et, et, mt[:, mpos[s]],
                                                    op=OP.mult)
                        ets[s] = et

                    # previous tile's out-projection, one feature chunk per
                    # head: fills the PE while Scalar works on our exps
                    if prev_ot is not None:
                        outproj_chunk(prev_ot, qsl_off - QT, h)
                    for s in range(min(LOOKAHEAD, n_slots)):
                        emit_score(s)
                    # previous head's normalization, pipelined behind our
                    # prologue so the PE never waits on its denominator
                    if pending is not None:
                        finish_norm(pending)
                    for s in range(n_slots):
                        if s + LOOKAHEAD < n_slots:
                            emit_score(s + LOOKAHEAD)
                        et = ets.pop(s)
                        nc.tensor.matmul(
                            psout,
                            lhsT=vsl[:, s],
                            rhs=et,
                            start=(s == 0),
                            stop=(s == n_slots - 1),
                        )
                        # denominator accumulation off the PE: two chains
                        # partitioned in time — GpSimd takes the early
                        # slots, DVE the late ones (less SBUF-port overlap)
                        half = n_slots // 2
                        if s == 0:
                            nc.vector.tensor_copy(_r(esum_b), et)
                        elif s < half:
                            nc.gpsimd.tensor_tensor(_r(esum_b), esum_b, et,
                                                    op=OP.add)
                        elif s == half:
                            nc.vector.tensor_copy(_r(esum), et)
                        else:
                            nc.vector.tensor_tensor(_r(esum), esum, et,
                                                    op=OP.add)
                    nc.vector.tensor_tensor(_r(esum), esum, esum_b, op=OP.add)
                    pending = (psout, esum, h)
                finish_norm(pending)
                prev_ot = ot_res

            # ---- out-projection for the last q tile ----
            for e in range(NH):
                outproj_chunk(prev_ot, (NQ - 1) * QT, e)

    nc.compile()
    return nc


# --------------------------------------------------------------------------
# Host-side prep and driver
# --------------------------------------------------------------------------

_PERMS = {0: (0, 1, 2, 3), 1: (1, 0, 3, 2)}


def make_host_data(x, w_in, w_out, q_gamma, q_beta, k_gamma, k_beta):
    """Build per-core in_maps (list of dicts) + assembly metadata."""
    import ml_dtypes
    bf16 = ml_dtypes.bfloat16

    B = x.shape[0]
    n_cores = 2 * B

    w64 = np.asarray(w_in, np.float64)
    wq = w64[0:D]
    wk = w64[D:2 * D]
    wv = w64[2 * D:3 * D]
    wq_c = wq - wq.mean(axis=0, keepdims=True)
    wk_c = wk - wk.mean(axis=0, keepdims=True)
    wqkT2 = np.concatenate([wq_c.T, wk_c.T], axis=1)   # [D, 2D]
    wqk_t = np.ascontiguousarray(
        wqkT2.reshape(DC, P, 2 * NH, P).transpose(2, 1, 0, 3)
    ).astype(bf16)
    wvT = wv.T  # [D(d), D(f)]
    wv_t = np.ascontiguousarray(
        wvT.reshape(DC, P, NH, P)
    ).astype(bf16)
    woT = np.asarray(w_out, np.float64).T  # [D(hfeat), D(eout)]
    wo_t = np.ascontiguousarray(
        woT.reshape(NH, P, NH, P).transpose(2, 1, 0, 3)
    ).astype(bf16)

    inv = 1.0 / (10000.0 ** (np.arange(0, HD, 2, dtype=np.float64) / HD))
    tpos = np.arange(S, dtype=np.float64)
    fr = np.outer(tpos, inv)
    emb = np.concatenate([fr, fr], axis=-1)  # [S, HD]
    cosT = np.cos(emb).T  # [HD, S]
    sinT = np.sin(emb).T

    h2 = HD // 2
    rotmT = np.zeros((P, P), np.float32)
    for p in range(h2):
        rotmT[p + h2, p] = -1.0
    for p in range(h2, HD):
        rotmT[p - h2, p] = 1.0
    rotm = rotmT.astype(bf16)

    scale = 1.0 / math.sqrt(HD)
    gq_a = np.ascontiguousarray(
        (np.asarray(q_gamma, np.float64) * scale).reshape(NH, P).T
    ).astype(np.float32)
    bq_a = np.ascontiguousarray(
        (np.asarray(q_beta, np.float64) * scale).reshape(NH, P).T
    ).astype(np.float32)
    gk_a = np.ascontiguousarray(
        np.asarray(k_gamma, np.float32).reshape(NH, P).T
    )
    bk_a = np.ascontiguousarray(
        np.asarray(k_beta, np.float32).reshape(NH, P).T
    )
    onesc = np.ones((P, 1), np.float32)
    onesr = np.ones((1, P), np.float32)

    xb_T = {}
    in_maps = []
    meta = []
    for c in range(n_cores):
        b = c // 2
        r = c % 2
        perm = _PERMS[r]
        ptok = np.concatenate(
            [np.arange(pb * QT, (pb + 1) * QT) for pb in perm]
        )
        if b not in xb_T:
            xb_T[b] = np.ascontiguousarray(
                np.asarray(x[b], np.float32).T
            )  # [D, S] f32
        xT = np.ascontiguousarray(xb_T[b][:, ptok]).astype(bf16)
        cosp = np.ascontiguousarray(cosT[:, ptok]).astype(bf16)
        sinp = np.ascontiguousarray(sinT[:, ptok]).astype(bf16)

        # masks in PERMUTED kv space; q slabs at permuted positions Q_POS.
        # Elementwise masks only on diagonal slots; other slots use the
        # per-row exp bias: -EXP_BIAS for fully valid rows, BIAS_INVALID
        # for fully invalid rows.
        masks = np.zeros([NQ, P, MAXM, QT], np.float32)
        biast = np.full([P, NQ, 16], -EXP_BIAS, np.float32)
        for t in range(NQ):
            gq_tok = ptok[Q_POS[t] * QT + np.arange(QT)]
            gq_max = gq_tok.max()
            for mi, kc in enumerate(MASKED[t]):
                gkv = ptok[kc * P + np.arange(P)]
                masks[t, :, mi, :] = (
                    gkv[:, None] <= gq_tok[None, :]
                ).astype(np.float32)
            for kc in range(16):
                gkv = ptok[kc * P + np.arange(P)]
                biast[:, t, kc] = np.where(gkv <= gq_max, -EXP_BIAS,
                                           BIAS_INVALID)
        masks = masks.astype(bf16)

        qtok = np.concatenate(
            [np.arange(perm[pq] * QT, (perm[pq] + 1) * QT) for pq in Q_POS]
        )
        in_maps.append(dict(
            xT=xT, wqk=wqk_t, wv=wv_t, wo=wo_t,
            cos=cosp, sin=sinp,
            gq=gq_a, bq=bq_a, gk=gk_a, bk=bk_a, masks=masks,
            biast=biast, onesc=onesc, onesr=onesr, rotm=rotm,
        ))
        meta.append(dict(b=b, qtok=qtok))
    return in_maps, meta


_PROGRAM_CACHE = {}


def _get_program():
    if "full" not in _PROGRAM_CACHE:
        _PROGRAM_CACHE["full"] = build_program()
    return _PROGRAM_CACHE["full"]


def run_full(x, w_in, w_out, q_gamma, q_beta, k_gamma, k_beta,
             trace=False):
    from concourse.bass_utils import run_bass_kernel_spmd

    B = x.shape[0]
    n_cores = 2 * B
    in_maps, meta = make_host_data(
        x, w_in, w_out, q_gamma, q_beta, k_gamma, k_beta,
    )
    nc = _get_program()
    res = run_bass_kernel_spmd(
        nc, in_maps, core_ids=list(range(n_cores)), trace=trace,
    )
    out = np.empty((B, S, D), np.float32)
    for c in range(n_cores):
        o = res.results[c]["out"]  # [D, NQTOK]
        out[meta[c]["b"], meta[c]["qtok"], :] = o.T
    return out, res


def kernel(x, w_in, w_out, q_gamma, q_beta, k_gamma, k_beta, n_heads=16,
           **_ignored):
    x = np.asarray(x, np.float32)
    assert int(np.asarray(n_heads)) * HD == x.shape[-1]
    out, _ = run_full(
        np.asarray(x, np.float32),
        np.asarray(w_in, np.float32),
        np.asarray(w_out, np.float32),
        np.asarray(q_gamma, np.float32),
        np.asarray(q_beta, np.float32),
        np.asarray(k_gamma, np.float32),
        np.asarray(k_beta, np.float32),
    )
    return out
